# revision 15
# baseline (speedup 1.0000x reference)
"""Packed ragged gather (tgt_cache_loc / to_free_slots) on 8 Trainium2 cores.

Strategy: data-parallel over batch rows (4096 rows/core, 32 rows per SBUF
partition). Host computes exclusive-prefix offsets. Device: dense-load the
core's rows into SBUF; for each of the 32 row-ranks k, one DVE op builds a
masked copy of the 128 rank-k rows (head-mask iota<L for tgt, tail-mask
iota>=V-F for free); a gpsimd indirect scatter-add DMA writes each masked
384-elem window into the pre-zeroed packed output at that row's exact packed
offset (one offset per partition). Window paddings are zeros, and windows of
consecutive rows overlap only on zeros, so the adds commute and no ordering
is needed. Host slices off pad regions and concatenates the 8 core segments.
"""

import numpy as np

import concourse.bacc as bacc
import concourse.bass as bass
import concourse.mybir as mybir
import concourse.tile as tile
from concourse import bass_utils

P = 128
NCORES = 8
V = 384


def kernel(accept_length, to_free_num_slots, out_cache_loc):
    B, V_ = out_cache_loc.shape
    assert V_ == V
    R = B // NCORES          # rows per core
    K = R // P               # rows per partition (row-ranks)
    out_dtype = out_cache_loc.dtype
    src_all = np.ascontiguousarray(out_cache_loc.astype(np.int32))

    L = accept_length.astype(np.int64) + 1
    F = to_free_num_slots.astype(np.int64)
    tgt_off = np.cumsum(L) - L
    free_off = np.cumsum(F) - F
    total_tgt = int(L.sum())
    total_free = int(F.sum())

    core_T = []
    in_maps = []
    iota = np.broadcast_to(np.arange(V, dtype=np.int32), (P, V)).copy()
    for c in range(NCORES):
        sl = slice(c * R, (c + 1) * R)
        Lc = L[sl].reshape(P, K)
        Fc = F[sl].reshape(P, K)
        to = (tgt_off[sl] - tgt_off[c * R]).reshape(P, K)
        fo = (free_off[sl] - free_off[c * R]).reshape(P, K)
        in_maps.append(
            {
                "src": src_all[sl].reshape(P, K * V),
                "thr_t": Lc.astype(np.int32),
                "thr_f": (V - Fc).astype(np.int32),
                "idx_t": to.astype(np.int32),
                "idx_f": (fo + Fc).astype(np.int32),  # = FRONT + fo - (V-F)
                "iota": iota,
            }
        )
        core_T.append((int(Lc.sum()), int(Fc.sum())))

    maxT = max(t for t, _ in core_T)
    maxFr = max(f for _, f in core_T)
    len_t = maxT + V
    len_f = V + maxFr + V  # front pad V (windows may start before data) + back pad

    # --- Device program (identical on all cores) ---
    nc = bacc.Bacc("TRN2", target_bir_lowering=False, debug=False)
    src_t = nc.dram_tensor("src", [P, K * V], mybir.dt.int32, kind="ExternalInput")
    thr_t = nc.dram_tensor("thr_t", [P, K], mybir.dt.int32, kind="ExternalInput")
    thr_f = nc.dram_tensor("thr_f", [P, K], mybir.dt.int32, kind="ExternalInput")
    idx_t = nc.dram_tensor("idx_t", [P, K], mybir.dt.int32, kind="ExternalInput")
    idx_f = nc.dram_tensor("idx_f", [P, K], mybir.dt.int32, kind="ExternalInput")
    iota_t = nc.dram_tensor("iota", [P, V], mybir.dt.int32, kind="ExternalInput")
    # Several buffers per output: scatters rotate across them, giving many
    # independent WAW chains so Tile never stalls on a same-tensor
    # predecessor's completion. Host sums the buffers (windows overlapping
    # across buffers carry disjoint real data, zeros elsewhere; within a
    # buffer the DMA CCE-add accumulates).
    NSPLIT = 2
    outs = {}
    for okey, ln in (("t", len_t), ("f", len_f)):
        for s in range(NSPLIT):
            outs[(okey, s)] = nc.dram_tensor(
                f"out_{okey}{s}", [ln], mybir.dt.int32, kind="ExternalOutput"
            )

    with tile.TileContext(nc) as tc:
        with (
            tc.tile_pool(name="persist", bufs=1) as ppool,
            tc.tile_pool(name="mask", bufs=24) as mpool,
        ):
            # load order tuned so the first masks/scatters can start earliest:
            # iota + thresholds, then src chunk 0, then scatter indices, rest.
            iob = ppool.tile([P, V], mybir.dt.int32, tag="iota")
            nc.sync.dma_start(out=iob[:], in_=iota_t.ap())
            small = {}
            for name, t in (("thr_t", thr_t), ("thr_f", thr_f)):
                st = ppool.tile([P, K], mybir.dt.int32, tag=name)
                nc.sync.dma_start(out=st[:], in_=t.ap())
                small[name] = st
            srcb = ppool.tile([P, K * V], mybir.dt.int32, tag="src")
            NCH = 8
            cw = (K * V) // NCH
            nc.sync.dma_start(out=srcb[:, :cw], in_=src_t.ap()[:, :cw])
            for name, t in (("idx_t", idx_t), ("idx_f", idx_f)):
                st = ppool.tile([P, K], mybir.dt.int32, tag=name)
                nc.sync.dma_start(out=st[:], in_=t.ap())
                small[name] = st
            for i in range(1, NCH):
                nc.sync.dma_start(
                    out=srcb[:, i * cw : (i + 1) * cw],
                    in_=src_t.ap()[:, i * cw : (i + 1) * cw],
                )

            for k in range(K):
                row = srcb[:, k * V : (k + 1) * V]
                for okey, op0 in (("t", mybir.AluOpType.is_lt), ("f", mybir.AluOpType.is_ge)):
                    ot = outs[(okey, k % NSPLIT)]
                    m = mpool.tile([P, V], mybir.dt.int32, tag=f"m{okey}")
                    nc.vector.scalar_tensor_tensor(
                        out=m[:],
                        in0=iob[:],
                        scalar=small[f"thr_{okey}"][:, k : k + 1],
                        in1=row,
                        op0=op0,
                        op1=mybir.AluOpType.mult,
                    )
                    nc.gpsimd.indirect_dma_start(
                        out=ot.ap()[:, None],
                        out_offset=bass.IndirectOffsetOnAxis(
                            ap=small[f"idx_{okey}"][:, k : k + 1], axis=0
                        ),
                        in_=m[:],
                        in_offset=None,
                        compute_op=mybir.AluOpType.add,
                    )

    nc.compile()
    res = bass_utils.run_bass_kernel_spmd(nc, in_maps, core_ids=list(range(NCORES)))
    if res.exec_time_ns is not None:
        print(f"HW exec time: {res.exec_time_ns} ns")

    tgt = np.empty(total_tgt, np.int32)
    fre = np.empty(total_free, np.int32)
    pt = pf = 0
    for c in range(NCORES):
        tc_, fc_ = core_T[c]
        rc = res.results[c]
        tsum = rc["out_t0"].copy()
        fsum = rc["out_f0"].copy()
        for s in range(1, 2):
            tsum += rc[f"out_t{s}"]
            fsum += rc[f"out_f{s}"]
        tgt[pt : pt + tc_] = tsum[:tc_]
        fre[pf : pf + fc_] = fsum[V : V + fc_]
        pt += tc_
        pf += fc_
    return tgt.astype(out_dtype), fre.astype(out_dtype)


# revision 17
# speedup vs baseline: 1.0742x; 1.0742x over previous
"""Packed ragged gather (tgt_cache_loc / to_free_slots) on 8 Trainium2 cores.

Strategy: data-parallel over batch rows (4096 rows/core, 32 rows per SBUF
partition). Host computes exclusive-prefix offsets. Device: dense-load the
core's rows into SBUF; for each of the 32 row-ranks k, one DVE op builds a
masked copy of the 128 rank-k rows (head-mask iota<L for tgt, tail-mask
iota>=V-F for free); a gpsimd indirect scatter-add DMA writes each masked
384-elem window into the pre-zeroed packed output at that row's exact packed
offset (one offset per partition). Window paddings are zeros, and windows of
consecutive rows overlap only on zeros, so the adds commute and no ordering
is needed. Host slices off pad regions and concatenates the 8 core segments.
"""

import numpy as np

import concourse.bacc as bacc
import concourse.bass as bass
import concourse.mybir as mybir
import concourse.tile as tile
from concourse import bass_utils

P = 128
NCORES = 8
V = 384


def kernel(accept_length, to_free_num_slots, out_cache_loc):
    B, V_ = out_cache_loc.shape
    assert V_ == V
    R = B // NCORES          # rows per core
    K = R // P               # rows per partition (row-ranks)
    out_dtype = out_cache_loc.dtype
    src_all = np.ascontiguousarray(out_cache_loc.astype(np.int32))

    L = accept_length.astype(np.int64) + 1
    F = to_free_num_slots.astype(np.int64)
    tgt_off = np.cumsum(L) - L
    free_off = np.cumsum(F) - F
    total_tgt = int(L.sum())
    total_free = int(F.sum())

    core_T = []
    in_maps = []
    iota = np.broadcast_to(np.arange(V, dtype=np.int32), (P, V)).copy()
    for c in range(NCORES):
        sl = slice(c * R, (c + 1) * R)
        Lc = L[sl].reshape(P, K)
        Fc = F[sl].reshape(P, K)
        to = (tgt_off[sl] - tgt_off[c * R]).reshape(P, K)
        fo = (free_off[sl] - free_off[c * R]).reshape(P, K)
        in_maps.append(
            {
                "src": src_all[sl].reshape(P, K * V),
                "thr_t": Lc.astype(np.int32),
                "thr_f": (V - Fc).astype(np.int32),
                "idx_t": to.astype(np.int32),
                "idx_f": (fo + Fc).astype(np.int32),  # = FRONT + fo - (V-F)
                "iota": iota,
            }
        )
        core_T.append((int(Lc.sum()), int(Fc.sum())))

    maxT = max(t for t, _ in core_T)
    maxFr = max(f for _, f in core_T)
    len_t = maxT + V
    len_f = V + maxFr + V  # front pad V (windows may start before data) + back pad

    # --- Device program (identical on all cores) ---
    nc = bacc.Bacc("TRN2", target_bir_lowering=False, debug=False)
    src_t = nc.dram_tensor("src", [P, K * V], mybir.dt.int32, kind="ExternalInput")
    thr_t = nc.dram_tensor("thr_t", [P, K], mybir.dt.int32, kind="ExternalInput")
    thr_f = nc.dram_tensor("thr_f", [P, K], mybir.dt.int32, kind="ExternalInput")
    idx_t = nc.dram_tensor("idx_t", [P, K], mybir.dt.int32, kind="ExternalInput")
    idx_f = nc.dram_tensor("idx_f", [P, K], mybir.dt.int32, kind="ExternalInput")
    iota_t = nc.dram_tensor("iota", [P, V], mybir.dt.int32, kind="ExternalInput")
    # Several buffers per output: scatters rotate across them, giving many
    # independent WAW chains so Tile never stalls on a same-tensor
    # predecessor's completion. Host sums the buffers (windows overlapping
    # across buffers carry disjoint real data, zeros elsewhere; within a
    # buffer the DMA CCE-add accumulates).
    NSPLIT = 2
    outs = {}
    for okey, ln in (("t", len_t), ("f", len_f)):
        for s in range(NSPLIT):
            outs[(okey, s)] = nc.dram_tensor(
                f"out_{okey}{s}", [ln], mybir.dt.int32, kind="ExternalOutput"
            )

    with tile.TileContext(nc) as tc:
        with (
            tc.tile_pool(name="persist", bufs=1) as ppool,
            tc.tile_pool(name="mask", bufs=8) as mpool,
        ):
            iob = ppool.tile([P, V], mybir.dt.int32, tag="iota")
            small = {}
            for name, t in (("thr_t", thr_t), ("thr_f", thr_f), ("idx_t", idx_t), ("idx_f", idx_f)):
                st = ppool.tile([P, K], mybir.dt.int32, tag=name)
                nc.sync.dma_start(out=st[:], in_=t.ap())
                small[name] = st
            nc.sync.dma_start(out=iob[:], in_=iota_t.ap())
            # big dense load, chunked for pipelining
            srcb = ppool.tile([P, K * V], mybir.dt.int32, tag="src")
            NCH = 8
            cw = (K * V) // NCH
            for i in range(NCH):
                nc.sync.dma_start(
                    out=srcb[:, i * cw : (i + 1) * cw],
                    in_=src_t.ap()[:, i * cw : (i + 1) * cw],
                )

            for k in range(K):
                row = srcb[:, k * V : (k + 1) * V]
                for okey, op0 in (("t", mybir.AluOpType.is_lt), ("f", mybir.AluOpType.is_ge)):
                    ot = outs[(okey, k % NSPLIT)]
                    m = mpool.tile([P, V], mybir.dt.int32, tag=f"m{okey}")
                    nc.vector.scalar_tensor_tensor(
                        out=m[:],
                        in0=iob[:],
                        scalar=small[f"thr_{okey}"][:, k : k + 1],
                        in1=row,
                        op0=op0,
                        op1=mybir.AluOpType.mult,
                    )
                    nc.gpsimd.indirect_dma_start(
                        out=ot.ap()[:, None],
                        out_offset=bass.IndirectOffsetOnAxis(
                            ap=small[f"idx_{okey}"][:, k : k + 1], axis=0
                        ),
                        in_=m[:],
                        in_offset=None,
                        compute_op=mybir.AluOpType.add,
                    )

    nc.compile()
    res = bass_utils.run_bass_kernel_spmd(nc, in_maps, core_ids=list(range(NCORES)))
    if res.exec_time_ns is not None:
        print(f"HW exec time: {res.exec_time_ns} ns")

    tgt = np.empty(total_tgt, np.int32)
    fre = np.empty(total_free, np.int32)
    pt = pf = 0
    for c in range(NCORES):
        tc_, fc_ = core_T[c]
        rc = res.results[c]
        tsum = rc["out_t0"].copy()
        fsum = rc["out_f0"].copy()
        for s in range(1, 2):
            tsum += rc[f"out_t{s}"]
            fsum += rc[f"out_f{s}"]
        tgt[pt : pt + tc_] = tsum[:tc_]
        fre[pf : pf + fc_] = fsum[V : V + fc_]
        pt += tc_
        pf += fc_
    return tgt.astype(out_dtype), fre.astype(out_dtype)


# revision 18
# speedup vs baseline: 1.0940x; 1.0185x over previous
"""Packed ragged gather (tgt_cache_loc / to_free_slots) on 8 Trainium2 cores.

Strategy: data-parallel over batch rows (4096 rows/core, 32 rows per SBUF
partition). Host computes exclusive-prefix offsets. Device: dense-load the
core's rows into SBUF; for each of the 32 row-ranks k, one DVE op builds a
masked copy of the 128 rank-k rows (head-mask iota<L for tgt, tail-mask
iota>=V-F for free); a gpsimd indirect scatter-add DMA writes each masked
384-elem window into the pre-zeroed packed output at that row's exact packed
offset (one offset per partition). Window paddings are zeros, and windows of
consecutive rows overlap only on zeros, so the adds commute and no ordering
is needed. Host slices off pad regions and concatenates the 8 core segments.
"""

import numpy as np

import concourse.bacc as bacc
import concourse.bass as bass
import concourse.mybir as mybir
import concourse.tile as tile
from concourse import bass_utils

P = 128
NCORES = 8
V = 384


def kernel(accept_length, to_free_num_slots, out_cache_loc):
    B, V_ = out_cache_loc.shape
    assert V_ == V
    R = B // NCORES          # rows per core
    K = R // P               # rows per partition (row-ranks)
    out_dtype = out_cache_loc.dtype
    src_all = np.ascontiguousarray(out_cache_loc.astype(np.int32))

    L = accept_length.astype(np.int64) + 1
    F = to_free_num_slots.astype(np.int64)
    tgt_off = np.cumsum(L) - L
    free_off = np.cumsum(F) - F
    total_tgt = int(L.sum())
    total_free = int(F.sum())

    core_T = []
    in_maps = []
    iota = np.broadcast_to(np.arange(V, dtype=np.int32), (P, V)).copy()
    for c in range(NCORES):
        sl = slice(c * R, (c + 1) * R)
        Lc = L[sl].reshape(P, K)
        Fc = F[sl].reshape(P, K)
        to = (tgt_off[sl] - tgt_off[c * R]).reshape(P, K)
        fo = (free_off[sl] - free_off[c * R]).reshape(P, K)
        in_maps.append(
            {
                "src": src_all[sl].reshape(P, K * V),
                "thr_t": Lc.astype(np.int32),
                "thr_f": (V - Fc).astype(np.int32),
                "idx_t": to.astype(np.int32),
                "idx_f": (fo + Fc).astype(np.int32),  # = FRONT + fo - (V-F)
                "iota": iota,
            }
        )
        core_T.append((int(Lc.sum()), int(Fc.sum())))

    maxT = max(t for t, _ in core_T)
    maxFr = max(f for _, f in core_T)
    len_t = maxT + V
    len_f = V + maxFr + V  # front pad V (windows may start before data) + back pad

    # --- Device program (identical on all cores) ---
    nc = bacc.Bacc("TRN2", target_bir_lowering=False, debug=False)
    src_t = nc.dram_tensor("src", [P, K * V], mybir.dt.int32, kind="ExternalInput")
    thr_t = nc.dram_tensor("thr_t", [P, K], mybir.dt.int32, kind="ExternalInput")
    thr_f = nc.dram_tensor("thr_f", [P, K], mybir.dt.int32, kind="ExternalInput")
    idx_t = nc.dram_tensor("idx_t", [P, K], mybir.dt.int32, kind="ExternalInput")
    idx_f = nc.dram_tensor("idx_f", [P, K], mybir.dt.int32, kind="ExternalInput")
    iota_t = nc.dram_tensor("iota", [P, V], mybir.dt.int32, kind="ExternalInput")
    # Several buffers per output: scatters rotate across them, giving many
    # independent WAW chains so Tile never stalls on a same-tensor
    # predecessor's completion. Host sums the buffers (windows overlapping
    # across buffers carry disjoint real data, zeros elsewhere; within a
    # buffer the DMA CCE-add accumulates).
    NSPLIT = 3
    outs = {}
    for okey, ln in (("t", len_t), ("f", len_f)):
        for s in range(NSPLIT):
            outs[(okey, s)] = nc.dram_tensor(
                f"out_{okey}{s}", [ln], mybir.dt.int32, kind="ExternalOutput"
            )

    with tile.TileContext(nc) as tc:
        with (
            tc.tile_pool(name="persist", bufs=1) as ppool,
            tc.tile_pool(name="mask", bufs=8) as mpool,
        ):
            iob = ppool.tile([P, V], mybir.dt.int32, tag="iota")
            small = {}
            for name, t in (("thr_t", thr_t), ("thr_f", thr_f), ("idx_t", idx_t), ("idx_f", idx_f)):
                st = ppool.tile([P, K], mybir.dt.int32, tag=name)
                nc.sync.dma_start(out=st[:], in_=t.ap())
                small[name] = st
            nc.sync.dma_start(out=iob[:], in_=iota_t.ap())
            # big dense load, chunked for pipelining
            srcb = ppool.tile([P, K * V], mybir.dt.int32, tag="src")
            NCH = 8
            cw = (K * V) // NCH
            for i in range(NCH):
                nc.sync.dma_start(
                    out=srcb[:, i * cw : (i + 1) * cw],
                    in_=src_t.ap()[:, i * cw : (i + 1) * cw],
                )

            for k in range(K):
                row = srcb[:, k * V : (k + 1) * V]
                for okey, op0 in (("t", mybir.AluOpType.is_lt), ("f", mybir.AluOpType.is_ge)):
                    ot = outs[(okey, k % NSPLIT)]
                    m = mpool.tile([P, V], mybir.dt.int32, tag=f"m{okey}")
                    nc.vector.scalar_tensor_tensor(
                        out=m[:],
                        in0=iob[:],
                        scalar=small[f"thr_{okey}"][:, k : k + 1],
                        in1=row,
                        op0=op0,
                        op1=mybir.AluOpType.mult,
                    )
                    nc.gpsimd.indirect_dma_start(
                        out=ot.ap()[:, None],
                        out_offset=bass.IndirectOffsetOnAxis(
                            ap=small[f"idx_{okey}"][:, k : k + 1], axis=0
                        ),
                        in_=m[:],
                        in_offset=None,
                        compute_op=mybir.AluOpType.add,
                    )

    nc.compile()
    res = bass_utils.run_bass_kernel_spmd(nc, in_maps, core_ids=list(range(NCORES)))
    if res.exec_time_ns is not None:
        print(f"HW exec time: {res.exec_time_ns} ns")

    tgt = np.empty(total_tgt, np.int32)
    fre = np.empty(total_free, np.int32)
    pt = pf = 0
    for c in range(NCORES):
        tc_, fc_ = core_T[c]
        rc = res.results[c]
        tsum = rc["out_t0"].copy()
        fsum = rc["out_f0"].copy()
        for s in range(1, 3):
            tsum += rc[f"out_t{s}"]
            fsum += rc[f"out_f{s}"]
        tgt[pt : pt + tc_] = tsum[:tc_]
        fre[pf : pf + fc_] = fsum[V : V + fc_]
        pt += tc_
        pf += fc_
    return tgt.astype(out_dtype), fre.astype(out_dtype)


# revision 21
# speedup vs baseline: 1.0947x; 1.0006x over previous
"""Packed ragged gather (tgt_cache_loc / to_free_slots) on 8 Trainium2 cores.

Strategy: data-parallel over batch rows (4096 rows/core, 32 rows per SBUF
partition). Host computes exclusive-prefix offsets. Device: dense-load the
core's rows into SBUF; for each of the 32 row-ranks k, one DVE op builds a
masked copy of the 128 rank-k rows (head-mask iota<L for tgt, tail-mask
iota>=V-F for free); a gpsimd indirect scatter-add DMA writes each masked
384-elem window into the pre-zeroed packed output at that row's exact packed
offset (one offset per partition). Window paddings are zeros, and windows of
consecutive rows overlap only on zeros, so the adds commute and no ordering
is needed. Host slices off pad regions and concatenates the 8 core segments.
"""

import numpy as np

import concourse.bacc as bacc
import concourse.bass as bass
import concourse.mybir as mybir
import concourse.tile as tile
from concourse import bass_utils

P = 128
NCORES = 8
V = 384


def kernel(accept_length, to_free_num_slots, out_cache_loc):
    B, V_ = out_cache_loc.shape
    assert V_ == V
    R = B // NCORES          # rows per core
    K = R // P               # rows per partition (row-ranks)
    out_dtype = out_cache_loc.dtype
    src_all = np.ascontiguousarray(out_cache_loc.astype(np.int32))

    L = accept_length.astype(np.int64) + 1
    F = to_free_num_slots.astype(np.int64)
    tgt_off = np.cumsum(L) - L
    free_off = np.cumsum(F) - F
    total_tgt = int(L.sum())
    total_free = int(F.sum())

    core_T = []
    in_maps = []
    iota = np.broadcast_to(np.arange(V, dtype=np.int32), (P, V)).copy()
    for c in range(NCORES):
        sl = slice(c * R, (c + 1) * R)
        Lc = L[sl].reshape(P, K)
        Fc = F[sl].reshape(P, K)
        to = (tgt_off[sl] - tgt_off[c * R]).reshape(P, K)
        fo = (free_off[sl] - free_off[c * R]).reshape(P, K)
        in_maps.append(
            {
                "src": src_all[sl].reshape(P, K * V),
                "thr_t": Lc.astype(np.int32),
                "thr_f": (V - Fc).astype(np.int32),
                "idx_t": to.astype(np.int32),
                "idx_f": (fo + Fc).astype(np.int32),  # = FRONT + fo - (V-F)
                "iota": iota,
            }
        )
        core_T.append((int(Lc.sum()), int(Fc.sum())))

    maxT = max(t for t, _ in core_T)
    maxFr = max(f for _, f in core_T)
    len_t = maxT + V
    len_f = V + maxFr + V  # front pad V (windows may start before data) + back pad

    # --- Device program (identical on all cores) ---
    nc = bacc.Bacc(
        "TRN2",
        target_bir_lowering=False,
        debug=False,
        dynamic_dma_scratch_size=65536,
    )
    src_t = nc.dram_tensor("src", [P, K * V], mybir.dt.int32, kind="ExternalInput")
    thr_t = nc.dram_tensor("thr_t", [P, K], mybir.dt.int32, kind="ExternalInput")
    thr_f = nc.dram_tensor("thr_f", [P, K], mybir.dt.int32, kind="ExternalInput")
    idx_t = nc.dram_tensor("idx_t", [P, K], mybir.dt.int32, kind="ExternalInput")
    idx_f = nc.dram_tensor("idx_f", [P, K], mybir.dt.int32, kind="ExternalInput")
    iota_t = nc.dram_tensor("iota", [P, V], mybir.dt.int32, kind="ExternalInput")
    # Several buffers per output: scatters rotate across them, giving many
    # independent WAW chains so Tile never stalls on a same-tensor
    # predecessor's completion. Host sums the buffers (windows overlapping
    # across buffers carry disjoint real data, zeros elsewhere; within a
    # buffer the DMA CCE-add accumulates).
    NSPLIT = 3
    outs = {}
    for okey, ln in (("t", len_t), ("f", len_f)):
        for s in range(NSPLIT):
            outs[(okey, s)] = nc.dram_tensor(
                f"out_{okey}{s}", [ln], mybir.dt.int32, kind="ExternalOutput"
            )

    with tile.TileContext(nc) as tc:
        with (
            tc.tile_pool(name="persist", bufs=1) as ppool,
            tc.tile_pool(name="mask", bufs=8) as mpool,
        ):
            iob = ppool.tile([P, V], mybir.dt.int32, tag="iota")
            small = {}
            for name, t in (("thr_t", thr_t), ("thr_f", thr_f), ("idx_t", idx_t), ("idx_f", idx_f)):
                st = ppool.tile([P, K], mybir.dt.int32, tag=name)
                nc.sync.dma_start(out=st[:], in_=t.ap())
                small[name] = st
            nc.sync.dma_start(out=iob[:], in_=iota_t.ap())
            # big dense load, chunked for pipelining
            srcb = ppool.tile([P, K * V], mybir.dt.int32, tag="src")
            NCH = 8
            cw = (K * V) // NCH
            for i in range(NCH):
                nc.sync.dma_start(
                    out=srcb[:, i * cw : (i + 1) * cw],
                    in_=src_t.ap()[:, i * cw : (i + 1) * cw],
                )

            for k in range(K):
                row = srcb[:, k * V : (k + 1) * V]
                for okey, op0 in (("t", mybir.AluOpType.is_lt), ("f", mybir.AluOpType.is_ge)):
                    ot = outs[(okey, k % NSPLIT)]
                    m = mpool.tile([P, V], mybir.dt.int32, tag=f"m{okey}")
                    nc.vector.scalar_tensor_tensor(
                        out=m[:],
                        in0=iob[:],
                        scalar=small[f"thr_{okey}"][:, k : k + 1],
                        in1=row,
                        op0=op0,
                        op1=mybir.AluOpType.mult,
                    )
                    nc.gpsimd.indirect_dma_start(
                        out=ot.ap()[:, None],
                        out_offset=bass.IndirectOffsetOnAxis(
                            ap=small[f"idx_{okey}"][:, k : k + 1], axis=0
                        ),
                        in_=m[:],
                        in_offset=None,
                        compute_op=mybir.AluOpType.add,
                    )

    nc.compile()
    res = bass_utils.run_bass_kernel_spmd(nc, in_maps, core_ids=list(range(NCORES)))
    if res.exec_time_ns is not None:
        print(f"HW exec time: {res.exec_time_ns} ns")

    tgt = np.empty(total_tgt, np.int32)
    fre = np.empty(total_free, np.int32)
    pt = pf = 0
    for c in range(NCORES):
        tc_, fc_ = core_T[c]
        rc = res.results[c]
        tsum = rc["out_t0"].copy()
        fsum = rc["out_f0"].copy()
        for s in range(1, 3):
            tsum += rc[f"out_t{s}"]
            fsum += rc[f"out_f{s}"]
        tgt[pt : pt + tc_] = tsum[:tc_]
        fre[pf : pf + fc_] = fsum[V : V + fc_]
        pt += tc_
        pf += fc_
    return tgt.astype(out_dtype), fre.astype(out_dtype)


# revision 23
# speedup vs baseline: 1.1138x; 1.0174x over previous
"""Packed ragged gather (tgt_cache_loc / to_free_slots) on 8 Trainium2 cores.

Strategy: data-parallel over batch rows (4096 rows/core, 32 rows per SBUF
partition). Host computes exclusive-prefix offsets. Device: dense-load the
core's rows into SBUF; for each of the 32 row-ranks k, one DVE op builds a
masked copy of the 128 rank-k rows (head-mask iota<L for tgt, tail-mask
iota>=V-F for free); a gpsimd indirect scatter-add DMA writes each masked
384-elem window into the pre-zeroed packed output at that row's exact packed
offset (one offset per partition). Window paddings are zeros, and windows of
consecutive rows overlap only on zeros, so the adds commute and no ordering
is needed. Host slices off pad regions and concatenates the 8 core segments.
"""

import numpy as np

import concourse.bacc as bacc
import concourse.bass as bass
import concourse.mybir as mybir
import concourse.tile as tile
from concourse import bass_utils

P = 128
NCORES = 8
V = 384


def kernel(accept_length, to_free_num_slots, out_cache_loc):
    B, V_ = out_cache_loc.shape
    assert V_ == V
    R = B // NCORES          # rows per core
    K = R // P               # rows per partition (row-ranks)
    out_dtype = out_cache_loc.dtype
    src_all = np.ascontiguousarray(out_cache_loc.astype(np.int32))

    L = accept_length.astype(np.int64) + 1
    F = to_free_num_slots.astype(np.int64)
    tgt_off = np.cumsum(L) - L
    free_off = np.cumsum(F) - F
    total_tgt = int(L.sum())
    total_free = int(F.sum())

    core_T = []
    in_maps = []
    iota = np.broadcast_to(np.arange(V, dtype=np.int32), (P, V)).copy()
    for c in range(NCORES):
        sl = slice(c * R, (c + 1) * R)
        Lc = L[sl].reshape(P, K)
        Fc = F[sl].reshape(P, K)
        to = (tgt_off[sl] - tgt_off[c * R]).reshape(P, K)
        fo = (free_off[sl] - free_off[c * R]).reshape(P, K)
        in_maps.append(
            {
                "src": src_all[sl].reshape(P, K * V),
                "thr_t": Lc.astype(np.int32),
                "thr_f": (V - Fc).astype(np.int32),
                "idx_t": to.astype(np.int32),
                "idx_f": (fo + Fc).astype(np.int32),  # = FRONT + fo - (V-F)
                "iota": iota,
            }
        )
        core_T.append((int(Lc.sum()), int(Fc.sum())))

    maxT = max(t for t, _ in core_T)
    maxFr = max(f for _, f in core_T)
    len_t = maxT + V
    len_f = V + maxFr + V  # front pad V (windows may start before data) + back pad

    # --- Device program (identical on all cores) ---
    nc = bacc.Bacc("TRN2", target_bir_lowering=False, debug=False)
    src_t = nc.dram_tensor("src", [P, K * V], mybir.dt.int32, kind="ExternalInput")
    thr_t = nc.dram_tensor("thr_t", [P, K], mybir.dt.int32, kind="ExternalInput")
    thr_f = nc.dram_tensor("thr_f", [P, K], mybir.dt.int32, kind="ExternalInput")
    idx_t = nc.dram_tensor("idx_t", [P, K], mybir.dt.int32, kind="ExternalInput")
    idx_f = nc.dram_tensor("idx_f", [P, K], mybir.dt.int32, kind="ExternalInput")
    iota_t = nc.dram_tensor("iota", [P, V], mybir.dt.int32, kind="ExternalInput")
    # Several buffers per output: scatters rotate across them, giving many
    # independent WAW chains so Tile never stalls on a same-tensor
    # predecessor's completion. Host sums the buffers (windows overlapping
    # across buffers carry disjoint real data, zeros elsewhere; within a
    # buffer the DMA CCE-add accumulates).
    NSPLIT = 3
    outs = {}
    for okey, ln in (("t", len_t), ("f", len_f)):
        for s in range(NSPLIT):
            outs[(okey, s)] = nc.dram_tensor(
                f"out_{okey}{s}", [ln], mybir.dt.int32, kind="ExternalOutput"
            )

    with tile.TileContext(nc) as tc:
        with (
            tc.tile_pool(name="persist", bufs=1) as ppool,
            tc.tile_pool(name="mask", bufs=8) as mpool,
        ):
            # load order: mask prerequisites (iota, thresholds, first src
            # chunk) land first so DVE masks and scatters start earliest
            iob = ppool.tile([P, V], mybir.dt.int32, tag="iota")
            nc.sync.dma_start(out=iob[:], in_=iota_t.ap())
            small = {}
            for name, t in (("thr_t", thr_t), ("thr_f", thr_f)):
                st = ppool.tile([P, K], mybir.dt.int32, tag=name)
                nc.sync.dma_start(out=st[:], in_=t.ap())
                small[name] = st
            srcb = ppool.tile([P, K * V], mybir.dt.int32, tag="src")
            NCH = 8
            cw = (K * V) // NCH
            nc.sync.dma_start(out=srcb[:, :cw], in_=src_t.ap()[:, :cw])
            for name, t in (("idx_t", idx_t), ("idx_f", idx_f)):
                st = ppool.tile([P, K], mybir.dt.int32, tag=name)
                nc.sync.dma_start(out=st[:], in_=t.ap())
                small[name] = st
            for i in range(1, NCH):
                nc.sync.dma_start(
                    out=srcb[:, i * cw : (i + 1) * cw],
                    in_=src_t.ap()[:, i * cw : (i + 1) * cw],
                )

            for k in range(K):
                row = srcb[:, k * V : (k + 1) * V]
                for okey, op0 in (("t", mybir.AluOpType.is_lt), ("f", mybir.AluOpType.is_ge)):
                    ot = outs[(okey, k % NSPLIT)]
                    m = mpool.tile([P, V], mybir.dt.int32, tag=f"m{okey}")
                    nc.vector.scalar_tensor_tensor(
                        out=m[:],
                        in0=iob[:],
                        scalar=small[f"thr_{okey}"][:, k : k + 1],
                        in1=row,
                        op0=op0,
                        op1=mybir.AluOpType.mult,
                    )
                    nc.gpsimd.indirect_dma_start(
                        out=ot.ap()[:, None],
                        out_offset=bass.IndirectOffsetOnAxis(
                            ap=small[f"idx_{okey}"][:, k : k + 1], axis=0
                        ),
                        in_=m[:],
                        in_offset=None,
                        compute_op=mybir.AluOpType.add,
                    )

    nc.compile()
    res = bass_utils.run_bass_kernel_spmd(nc, in_maps, core_ids=list(range(NCORES)))
    if res.exec_time_ns is not None:
        print(f"HW exec time: {res.exec_time_ns} ns")

    tgt = np.empty(total_tgt, np.int32)
    fre = np.empty(total_free, np.int32)
    pt = pf = 0
    for c in range(NCORES):
        tc_, fc_ = core_T[c]
        rc = res.results[c]
        tsum = rc["out_t0"].copy()
        fsum = rc["out_f0"].copy()
        for s in range(1, 3):
            tsum += rc[f"out_t{s}"]
            fsum += rc[f"out_f{s}"]
        tgt[pt : pt + tc_] = tsum[:tc_]
        fre[pf : pf + fc_] = fsum[V : V + fc_]
        pt += tc_
        pf += fc_
    return tgt.astype(out_dtype), fre.astype(out_dtype)
